# revision 12
# baseline (speedup 1.0000x reference)
"""Trainium2 Bass kernel for nn_AnchorExtractor (LoFTR coarse anchor NMS).

reference(conf_matrix, mconf, b_ids, i_ids, j_ids) returns
  (anchors [4,64,2,2] f32, conf_matrix [4,4096,4096] f32 passthrough).

Strategy (8 NeuronCores, SPMD):
  * conf_matrix passthrough is the memory-roofline part: reshape to
    [16384, 4096] f32 and shard 2048 rows per core; each core DMA-copies
    its 32 MiB shard DRAM->DRAM.
  * The match lists (mconf/b_ids/i_ids/j_ids) are scattered on host onto
    the per-batch [64,64] conf0 grid (last-write-wins, matching jax's
    CPU scatter semantics) -- per the sharding hint, the scattered conf0
    grid is the shardable representation.  Core c computes the full
    NMS + anchor selection for batch c%4 on-device:
      - 2x2 maxpool (pad right/bottom) via free-dim max + shift-matmul
      - local-max mask, raster-order ranking via triangular matmuls
      - anchor a picks the (a mod cnt)-th surviving cell; realized with
        exact integer-in-f32 arithmetic, one-hot row/col selection and
        a gather matmul.  No data-dependent control flow.
"""

import numpy as np

import concourse.bass as bass
import concourse.mybir as mybir
import concourse.tile as tile_mod
from concourse.tile import TileContext
from concourse.bass_utils import run_bass_kernel_spmd
from concourse.vector_clock import ScopedClock


def _split_drain_and_barrier(self, tick_clock, wait_clock):
    """TileContext kernel-tail drain, with the wait set split one-per-
    instruction: the walrus build behind this environment's compile path
    rejects any instruction carrying more than one sync wait."""
    nc = self.nc
    probe = nc.sync.nop(nofuse=True, hint="drain_waits")
    wait_clock.add_sem_waits(
        probe.ins, ScopedClock({None: tick_clock.global_clock}))
    si = probe.ins.sync_info
    waits = list(si.on_wait) if si is not None and si.on_wait else []
    upd = list(si.on_update) if si is not None and si.on_update else []
    last_wait = []
    if len(waits) > 1:
        probe.ins.sync_info = mybir.SyncInfo(on_wait=[waits[0]], on_update=upd)
        for w in waits[1:-1]:
            n = nc.sync.nop(nofuse=True, hint="drain_wait")
            n.ins.sync_info = mybir.SyncInfo(on_wait=[w], on_update=[])
        last_wait = [waits[-1]]
    drain_inst = nc.sync.drain()
    if last_wait:
        drain_inst.ins.sync_info = mybir.SyncInfo(on_wait=last_wait, on_update=[])
    nc.all_engine_barrier()
    assert self.sems is not None
    popped = nc._tile_sem_poison_stack.pop()
    assert popped is self._sem_poison
    nc.clear_and_free_semaphores(list(self.sems.allocated().values()))
    nc.all_engine_barrier()


tile_mod.TileContext._drain_and_barrier = _split_drain_and_barrier

BS = 4
H0 = W0 = 64
L0 = H0 * W0          # 4096
L1 = 4096
W1 = 64
A = 64                # anchor_num
N_CORES = 8
TOTAL_ROWS = BS * L0  # 16384 rows of the [16384, 4096] view
ROWS = TOTAL_ROWS // N_CORES  # 2048 per core
# 6 passthrough DMAs + smalls-load + anchors-store = 8 DMAs total, one per
# HWDGE sem lane: a 9th DMA would reuse a lane and acquire a second sync
# wait, which this walrus build rejects (one wait slot per instruction).
COPY_CHUNKS = 6

F32 = mybir.dt.float32
OP = mybir.AluOpType
AX = mybir.AxisListType

_NC_CACHE = {}


def _host_constants() -> np.ndarray:
    """[64, 321] f32 constant pack: U | ones | iotaF | identity | shiftT | iotaP."""
    n = 64
    k = np.arange(n)
    cst = np.zeros((n, 5 * n + 1), np.float32)
    cst[:, 0:n] = (k[:, None] < k[None, :])          # U[k,c] = k < c
    cst[:, n:2 * n] = 1.0                            # ones
    cst[:, 2 * n:3 * n] = k[None, :]                 # iotaF[p,c] = c
    cst[:, 3 * n:4 * n] = np.eye(n)                  # identity
    sh = np.zeros((n, n), np.float32)
    sh[np.arange(1, n), np.arange(0, n - 1)] = 1.0   # shiftT[r+1, r] = 1
    cst[:, 4 * n:5 * n] = sh
    cst[:, 5 * n] = k                                # iotaP[p] = p
    return cst


def _build_nc(rows: int = ROWS, copy_chunks: int = COPY_CHUNKS) -> bass.Bass:
    key = (rows, copy_chunks)
    if key in _NC_CACHE:
        return _NC_CACHE[key]

    nc = bass.Bass()
    cm_in = nc.declare_dram_parameter("cm_in", [rows, 4096], F32, isOutput=False)
    # conf | jhi | jlo | consts packed as one tensor -> one DMA -> one sem lane
    smalls = nc.declare_dram_parameter("smalls", [64, 513], F32, isOutput=False)
    cm_out = nc.declare_dram_parameter("cm_out", [rows, 4096], F32, isOutput=True)
    anc_out = nc.declare_dram_parameter("anc_out", [64, 4], F32, isOutput=True)

    # near-uniform row split into copy_chunks pieces
    base = rows // copy_chunks
    rem = rows % copy_chunks
    bounds = [0]
    for kk in range(copy_chunks):
        bounds.append(bounds[-1] + base + (1 if kk < rem else 0))

    with TileContext(nc) as tc:
        with (
            tc.tile_pool(name="sb", bufs=1) as sb,
            tc.tile_pool(name="ps", bufs=4, space="PSUM") as ps,
        ):
            # ---- load grids + constants with a single DMA, then funnel
            # through one DVE copy: PE instructions then wait on the DVE
            # semaphore only (the PE LoadWeights encoding has a single
            # sync-wait slot on this toolchain).
            sml0 = sb.tile([64, 513], F32, tag="sml0")
            nc.sync.dma_start(out=sml0[:], in_=smalls[:, :])
            sml = sb.tile([64, 513], F32, tag="sml")
            nc.vector.tensor_copy(out=sml[:], in_=sml0[:])

            # ---- memory-bound passthrough: DRAM -> DRAM, split across queues
            for kk in range(copy_chunks):
                nc.sync.dma_start(
                    out=cm_out[bounds[kk]:bounds[kk + 1], :],
                    in_=cm_in[bounds[kk]:bounds[kk + 1], :],
                )
            conf = sml[:, 0:64]
            jhi = sml[:, 64:128]
            jlo = sml[:, 128:192]
            U = sml[:, 192:256]
            ones = sml[:, 256:320]
            iotaF = sml[:, 320:384]
            ident = sml[:, 384:448]
            shiftT = sml[:, 448:512]
            iotaP = sml[:, 512:513]

            def mm(out_ps, lhsT, rhs):
                nc.tensor.matmul(out_ps, lhsT, rhs, start=True, stop=True)

            def tr(out_ps, in_ap):
                nc.tensor.transpose(out_ps, in_ap, ident)

            def ps_to_sb(shape, tag, fill):
                p = ps.tile(shape, F32, tag="pp")
                fill(p[:])
                s = sb.tile(shape, F32, tag=tag)
                nc.vector.tensor_copy(out=s[:], in_=p[:])
                return s

            # ---- 2x2 maxpool with (0,1,0,1) zero pad
            hmax = sb.tile([64, 64], F32, tag="hmax")
            nc.vector.tensor_tensor(
                out=hmax[:, 0:63], in0=sml[:, 0:63], in1=sml[:, 1:64], op=OP.max)
            nc.vector.tensor_copy(out=hmax[:, 63:64], in_=sml[:, 63:64])
            # hshift[r,:] = hmax[r+1,:], row 63 -> 0 (matmul with subdiagonal)
            hshift = ps_to_sb([64, 64], "hshift", lambda p: mm(p, shiftT, hmax[:]))
            nms = sb.tile([64, 64], F32, tag="nms")
            nc.vector.tensor_tensor(out=nms[:], in0=hmax[:], in1=hshift[:], op=OP.max)

            # ---- local-maxima mask M = (conf > 0) & (conf == nms)
            m1 = sb.tile([64, 64], F32, tag="m1")
            nc.vector.tensor_scalar(
                out=m1[:], in0=conf, scalar1=0.0, scalar2=None, op0=OP.is_gt)
            m2 = sb.tile([64, 64], F32, tag="m2")
            nc.vector.tensor_tensor(out=m2[:], in0=conf, in1=nms[:], op=OP.is_equal)
            M = sb.tile([64, 64], F32, tag="M")
            nc.vector.tensor_tensor(out=M[:], in0=m1[:], in1=m2[:], op=OP.mult)

            # ---- raster-order ranks
            rowsum = sb.tile([64, 1], F32, tag="rowsum")
            nc.vector.reduce_sum(out=rowsum[:], in_=M[:], axis=AX.X)
            maskT = ps_to_sb([64, 64], "maskT", lambda p: tr(p, M[:]))
            cex = ps_to_sb([64, 64], "cex", lambda p: mm(p, maskT[:], U))
            rowoff = ps_to_sb([64, 1], "rowoff", lambda p: mm(p, U, rowsum[:]))
            cnt = ps_to_sb([64, 1], "cnt", lambda p: mm(p, ones, rowsum[:]))

            # ---- t[a] = a mod max(cnt,1), exact integer arithmetic in f32
            cntc = sb.tile([64, 1], F32, tag="cntc")
            nc.vector.tensor_scalar_max(cntc[:], cnt[:], 1.0)
            mult = sb.tile([64, 1], F32, tag="mult")
            nc.vector.tensor_tensor(out=mult[:], in0=iotaP, in1=cntc[:], op=OP.mult)
            multT = ps_to_sb([64, 64], "multT",
                             lambda p: tr(p, mult[:].to_broadcast([64, 64])))
            cmp = sb.tile([64, 64], F32, tag="cmp")
            nc.vector.tensor_scalar(
                out=cmp[:], in0=multT[:], scalar1=iotaP, scalar2=None, op0=OP.is_le)
            ksum = sb.tile([64, 1], F32, tag="ksum")
            nc.vector.reduce_sum(out=ksum[:], in_=cmp[:], axis=AX.X)
            kfloor = sb.tile([64, 1], F32, tag="kfloor")
            nc.vector.tensor_scalar_add(kfloor[:], ksum[:], -1.0)
            kcnt = sb.tile([64, 1], F32, tag="kcnt")
            nc.vector.tensor_tensor(out=kcnt[:], in0=cntc[:], in1=kfloor[:], op=OP.mult)
            t = sb.tile([64, 1], F32, tag="t")
            nc.vector.tensor_tensor(out=t[:], in0=iotaP, in1=kcnt[:], op=OP.subtract)

            # ---- one-hot row selection: row r(a) with rowoff[r] <= t[a] < rowoff[r]+rowsum[r]
            rooT = ps_to_sb([64, 64], "rooT",
                            lambda p: tr(p, rowoff[:].to_broadcast([64, 64])))
            rosT = ps_to_sb([64, 64], "rosT",
                            lambda p: tr(p, rowsum[:].to_broadcast([64, 64])))
            geA = sb.tile([64, 64], F32, tag="geA")
            nc.vector.tensor_scalar(
                out=geA[:], in0=rooT[:], scalar1=t[:], scalar2=None, op0=OP.is_le)
            upper = sb.tile([64, 64], F32, tag="upper")
            nc.vector.tensor_tensor(out=upper[:], in0=rooT[:], in1=rosT[:], op=OP.add)
            ltB = sb.tile([64, 64], F32, tag="ltB")
            nc.vector.tensor_scalar(
                out=ltB[:], in0=upper[:], scalar1=t[:], scalar2=None, op0=OP.is_gt)
            rowsel = sb.tile([64, 64], F32, tag="rowsel")
            nc.vector.tensor_tensor(out=rowsel[:], in0=geA[:], in1=ltB[:], op=OP.mult)

            scr = sb.tile([64, 64], F32, tag="scr")
            r_a = sb.tile([64, 1], F32, tag="r_a")
            nc.vector.tensor_tensor(out=scr[:], in0=rowsel[:], in1=iotaF, op=OP.mult)
            nc.vector.reduce_sum(out=r_a[:], in_=scr[:], axis=AX.X)
            scr2 = sb.tile([64, 64], F32, tag="scr2")
            roff_a = sb.tile([64, 1], F32, tag="roff_a")
            nc.vector.tensor_tensor(out=scr2[:], in0=rowsel[:], in1=rooT[:], op=OP.mult)
            nc.vector.reduce_sum(out=roff_a[:], in_=scr2[:], axis=AX.X)
            tp = sb.tile([64, 1], F32, tag="tp")
            nc.vector.tensor_tensor(out=tp[:], in0=t[:], in1=roff_a[:], op=OP.subtract)

            # ---- gather row r(a) of [cex | M | jhi | jlo] with one matmul
            rsT = ps_to_sb([64, 64], "rsT", lambda p: tr(p, rowsel[:]))
            big = sb.tile([64, 256], F32, tag="big")
            nc.vector.tensor_copy(out=big[:, 0:64], in_=cex[:])
            nc.vector.tensor_copy(out=big[:, 64:128], in_=M[:])
            nc.vector.tensor_copy(out=big[:, 128:192], in_=jhi)
            nc.vector.tensor_copy(out=big[:, 192:256], in_=jlo)
            G = ps_to_sb([64, 256], "G", lambda p: mm(p, rsT[:], big[:]))

            # ---- one-hot column selection at rank tp within the gathered row
            eq = sb.tile([64, 64], F32, tag="eq")
            nc.vector.tensor_scalar(
                out=eq[:], in0=G[:, 0:64], scalar1=tp[:], scalar2=None, op0=OP.is_equal)
            colsel = sb.tile([64, 64], F32, tag="colsel")
            nc.vector.tensor_tensor(
                out=colsel[:], in0=eq[:], in1=G[:, 64:128], op=OP.mult)

            anc = sb.tile([64, 4], F32, tag="anc")
            scr3 = sb.tile([64, 64], F32, tag="scr3")
            nc.vector.tensor_tensor(out=scr3[:], in0=colsel[:], in1=iotaF, op=OP.mult)
            nc.vector.reduce_sum(out=anc[:, 1:2], in_=scr3[:], axis=AX.X)  # i % 64
            nc.vector.tensor_copy(out=anc[:, 0:1], in_=r_a[:])             # i // 64
            scr4 = sb.tile([64, 64], F32, tag="scr4")
            nc.vector.tensor_tensor(
                out=scr4[:], in0=colsel[:], in1=G[:, 128:192], op=OP.mult)
            nc.vector.reduce_sum(out=anc[:, 2:3], in_=scr4[:], axis=AX.X)  # j // 64
            scr5 = sb.tile([64, 64], F32, tag="scr5")
            nc.vector.tensor_tensor(
                out=scr5[:], in0=colsel[:], in1=G[:, 192:256], op=OP.mult)
            nc.vector.reduce_sum(out=anc[:, 3:4], in_=scr5[:], axis=AX.X)  # j % 64

            nc.sync.dma_start(out=anc_out[:, :], in_=anc[:])

    _NC_CACHE[key] = nc
    return nc


def _host_grids(mconf, b_ids, i_ids, j_ids):
    """Scatter match lists onto per-batch grids, last-write-wins."""
    conf_flat = np.zeros((BS, L0), np.float32)
    j_flat = np.zeros((BS, L0), np.int64)
    b = np.asarray(b_ids).astype(np.int64)
    i = np.asarray(i_ids).astype(np.int64)
    conf_flat[b, i] = np.asarray(mconf, np.float32)
    j_flat[b, i] = np.asarray(j_ids).astype(np.int64)
    conf_grid = conf_flat.reshape(BS, 64, 64)
    jhi = (j_flat // W1).astype(np.float32).reshape(BS, 64, 64)
    jlo = (j_flat % W1).astype(np.float32).reshape(BS, 64, 64)
    return conf_grid, jhi, jlo


def _run_spmd(inputs, trace=False, **kwargs):
    nc = _build_nc()
    conf_grid, jhi, jlo = _host_grids(
        inputs["mconf"], inputs["b_ids"], inputs["i_ids"], inputs["j_ids"])
    cm = np.ascontiguousarray(
        np.asarray(inputs["conf_matrix"], np.float32)).reshape(TOTAL_ROWS, 4096)
    cst = _host_constants()
    in_maps = []
    for c in range(N_CORES):
        b = c % BS
        sml = np.concatenate(
            [conf_grid[b].reshape(64, 64), jhi[b], jlo[b], cst], axis=1)
        in_maps.append({
            "cm_in": cm[c * ROWS:(c + 1) * ROWS],
            "smalls": np.ascontiguousarray(sml),
        })
    res = run_bass_kernel_spmd(
        nc, in_maps, core_ids=list(range(N_CORES)), trace=trace, **kwargs)
    return res


def _assemble(results):
    cm_out = np.concatenate(
        [results[c]["cm_out"] for c in range(N_CORES)], axis=0
    ).reshape(BS, L0, L1)
    anchors = np.zeros((BS, A, 2, 2), np.float32)
    for b in range(BS):
        anc = results[b]["anc_out"]  # [64, 4] = i//64, i%64, j//64, j%64
        anchors[b, :, 0, 0] = anc[:, 0]
        anchors[b, :, 0, 1] = anc[:, 1]
        anchors[b, :, 1, 0] = anc[:, 2]
        anchors[b, :, 1, 1] = anc[:, 3]
    return anchors, cm_out


def kernel(**inputs):
    try:
        res = _run_spmd(inputs, trace=False)
    except Exception:
        # one retry: the axon-tunneled devices occasionally report a
        # transient NRT exec-unit error right after another process's
        # teardown; a fresh dispatch succeeds.
        import time
        time.sleep(2.0)
        res = _run_spmd(inputs, trace=False)
    return _assemble(res.results)
